# revision 1
# baseline (speedup 1.0000x reference)
"""GCN forward on 8 TRN2 NeuronCores — v2.

Changes vs baseline:
  - Degree-balanced node->(core,group) assignment (snake by in-degree) so
    per-(core,group,chunk) segment counts are near-equal across cores.
  - Self-loops removed from the gather stream; their contribution is added
    in the epilogues from SBUF-resident local tables.
  - Packed block layout: per (supergroup, chunk), group segments are laid
    out back-to-back at fixed slot offsets (max-over-core length), so
    128-message blocks can span group boundaries. Boundary blocks issue one
    matmul per intersecting group using a separate one-hot column.

Math (reference.py): 2-layer GCN, N=100000, E=1600000, IN=HID=128, OUT=64,
symmetric normalization with self-loops:
  deg[t] = in_degree(t)+1 ; dinv = deg^-1/2
  table1[s] = dinv[s]*(x@W1)[s] ; y1[t] = relu(dinv[t]*(sum_e table1[row_e]
               + dinv[t]*table1[t]) + b1)
  table2[s] = dinv[s]*y1[s]     ; out[t] = (dinv[t]*(sum_e table2[row_e]
               + dinv[t]*table2[t]))@W2 + b2
"""

import sys

sys.path.insert(0, "/opt/trn_rl_repo")
import numpy as np
import ml_dtypes

import concourse.bass as bass
import concourse.mybir as mybir
import concourse.tile as tile
from concourse import bacc
from concourse.bass_utils import run_bass_kernel_spmd

F32 = mybir.dt.float32
BF16 = mybir.dt.bfloat16
I16 = mybir.dt.int16
I32 = mybir.dt.int32
AF = mybir.ActivationFunctionType
ALU = mybir.AluOpType

P = 128
N, E = 100000, 1600000
IN, HID, OUT = 128, 128, 64
NCORES = 8
PPC = 12544
NPAD = PPC * NCORES
NCHUNK = 4
CHUNK = NPAD // NCHUNK
NG = PPC // P  # 98 groups/core
SG = 8         # groups per supergroup
MAX_CALL_BLOCKS = 24


def host_prep(edge_index):
    row = np.asarray(edge_index[0], dtype=np.int64)
    col = np.asarray(edge_index[1], dtype=np.int64)
    deg = np.bincount(col, minlength=N).astype(np.int64) + 1

    # --- degree-balanced snake assignment of nodes to (core,group) bins ---
    NBINS = NCORES * NG
    order = np.argsort(-deg, kind="stable")
    bin_of = np.empty(N, np.int64)
    idx0 = 0
    rounds = 0
    while idx0 < N:
        take = min(NBINS, N - idx0)
        sel = order[idx0 : idx0 + take]
        seq = np.arange(take) if rounds % 2 == 0 else (NBINS - 1 - np.arange(take))
        bin_of[sel] = seq
        idx0 += take
        rounds += 1
    # slots within bins (max P per bin guaranteed: ceil(N/NBINS) <= P)
    order2 = np.argsort(bin_of, kind="stable")
    bcounts = np.bincount(bin_of, minlength=NBINS)
    bstarts = np.zeros(NBINS, np.int64)
    bstarts[1:] = np.cumsum(bcounts)[:-1]
    slot_of = np.empty(N, np.int64)
    slot_of[order2] = np.arange(N, dtype=np.int64) - np.repeat(bstarts, bcounts)
    assert bcounts.max() <= P
    core_of = bin_of // NG
    grp_of = bin_of % NG
    gids = core_of * PPC + grp_of * P + slot_of  # node -> padded slot

    # --- edges (no self-loops in the stream) ---
    grow = gids[row]
    gcol = gids[col]
    owner = gcol // PPC
    g_all = (gcol - owner * PPC) >> 7
    loc_all = (gcol - owner * PPC) & 127
    ch_all = grow // CHUNK
    lidx_all = (grow % CHUNK).astype(np.int64)

    NSEG = NG * NCHUNK
    counts = np.zeros((NCORES, NSEG), np.int64)
    per_core = []
    for c in range(NCORES):
        sel = owner == c
        g = g_all[sel]
        ch = ch_all[sel]
        o = np.lexsort((ch, g))
        seg = (g * NCHUNK + ch)[o]
        counts[c] = np.bincount(seg, minlength=NSEG)
        per_core.append((seg, lidx_all[sel][o], loc_all[sel][o]))

    seg_len = counts.max(axis=0)  # fixed per-(g,ch) slot allocation
    seg_len = np.maximum(seg_len, 1)

    # --- packed layout: per (sg, ch) concatenate group segments ---
    sgs = []
    g0 = 0
    while g0 < NG:
        sgs.append((g0, min(SG, NG - g0)))
        g0 += SG

    seg_off = np.zeros(NSEG, np.int64)   # message-slot offset of each segment
    calls = []                            # (sgi, ch, blk_off, nblk, scol_off, nscol)
    # scol list entries: (block_k_within_call, gl, colv_col_index, is_last)
    callcols = []
    tot_blocks = 0
    tot_scols = 0
    for sgi, (gs, ng) in enumerate(sgs):
        for ch in range(NCHUNK):
            # slots for this (sg, ch)
            L = 0
            seg_bounds = []
            for gl in range(ng):
                s = (gs + gl) * NCHUNK + ch
                seg_off[s] = tot_blocks * P + L
                seg_bounds.append((L, L + seg_len[s]))
                L += seg_len[s]
            nblk = -(-L // P)
            # columns: for each block, each group intersecting it
            cols = []
            for k in range(nblk):
                blo, bhi = k * P, (k + 1) * P
                for gl in range(ng):
                    slo, shi = seg_bounds[gl]
                    if slo < bhi and shi > blo:
                        is_first_seg = ch == 0 and slo >= blo
                        is_last_seg = ch == NCHUNK - 1 and shi <= bhi
                        cols.append((k, gl, is_first_seg, is_last_seg))
            # split into calls of <= MAX_CALL_BLOCKS blocks
            k0 = 0
            while k0 < nblk:
                nb = min(MAX_CALL_BLOCKS, nblk - k0)
                ccols = [
                    (k - k0, gl, first, last)
                    for (k, gl, first, last) in cols
                    if k0 <= k < k0 + nb
                ]
                calls.append(
                    (sgi, ch, tot_blocks + k0, nb, tot_scols, len(ccols))
                )
                callcols.append(ccols)
                tot_scols += len(ccols)
                k0 += nb
            tot_blocks += nblk
    TOTB = tot_blocks
    NSCOL = tot_scols

    # --- per-core idx / colv arrays ---
    idx_list, colv_list = [], []
    for c in range(NCORES):
        seg, lidx, loc = per_core[c]
        seg_start = np.zeros(NSEG, np.int64)
        seg_start[1:] = np.cumsum(counts[c])[:-1]
        rank = np.arange(seg.shape[0]) - seg_start[seg]
        dest = seg_off[seg] + rank
        idx_arr = np.zeros(TOTB * P, np.int16)
        idx_arr[dest] = lidx.astype(np.int16)
        idx_list.append(np.tile(idx_arr.reshape(-1, 16).T, (8, 1)).copy())

    for c in range(NCORES):
        seg, lidx, loc = per_core[c]
        seg_start = np.zeros(NSEG, np.int64)
        seg_start[1:] = np.cumsum(counts[c])[:-1]
        rank = np.arange(seg.shape[0]) - seg_start[seg]
        dest = seg_off[seg] + rank
        slot_gl = np.full(TOTB * P, -1, np.int64)
        slot_loc = np.full(TOTB * P, -1, np.int64)
        slot_gl[dest] = (seg // NCHUNK) % NG  # group index g (global)
        slot_loc[dest] = loc
        colv_arr = np.full((NSCOL, P), -1.0, np.float32)
        for ci, (sgi, ch, blk_off, nb, scol_off, nscol) in enumerate(calls):
            gs, ng = sgs[sgi]
            for j, (kk, gl, first, last) in enumerate(callcols[ci]):
                blk = blk_off + kk
                sl = slice(blk * P, (blk + 1) * P)
                m = slot_gl[sl] == gs + gl
                cv = colv_arr[scol_off + j]
                cv[m] = slot_loc[sl][m].astype(np.float32)
        colv_list.append(
            np.ascontiguousarray(colv_arr.T.astype(ml_dtypes.bfloat16))
        )

    degp = np.ones(NPAD, np.int32)
    degp[gids] = deg.astype(np.int32)
    dego_list = [
        np.ascontiguousarray(degp[c * PPC : (c + 1) * PPC].reshape(NG, P).T)
        for c in range(NCORES)
    ]

    sched = {
        "sgs": sgs,
        "calls": calls,
        "callcols": callcols,
        "TOTB": TOTB,
        "NSCOL": NSCOL,
        "gids": gids,
    }
    return sched, idx_list, colv_list, dego_list, gids


def build_kernel(sched, stage=9):
    sgs, calls, callcols = sched["sgs"], sched["calls"], sched["callcols"]
    TOTB, NSCOL = sched["TOTB"], sched["NSCOL"]

    nc = bacc.Bacc("TRN2", target_bir_lowering=False, num_devices=NCORES)
    xT = nc.dram_tensor("xT", [P, PPC], BF16, kind="ExternalInput")
    dego = nc.dram_tensor("dego", [P, NG], I32, kind="ExternalInput")
    W1 = nc.dram_tensor("W1", [IN, HID], F32, kind="ExternalInput")
    W2 = nc.dram_tensor("W2", [HID, OUT], F32, kind="ExternalInput")
    b1r = nc.dram_tensor("b1r", [P, HID], F32, kind="ExternalInput")
    b2r = nc.dram_tensor("b2r", [P, OUT], F32, kind="ExternalInput")
    iota = nc.dram_tensor("iota", [P, P], BF16, kind="ExternalInput")
    identd = nc.dram_tensor("identd", [P, P], BF16, kind="ExternalInput")
    idx = nc.dram_tensor("idx", [P, TOTB * 8], I16, kind="ExternalInput")
    colv = nc.dram_tensor("colv", [P, NSCOL], BF16, kind="ExternalInput")
    y = nc.dram_tensor("y", [P, NG, OUT], F32, kind="ExternalOutput")

    with tile.TileContext(nc) as tc:
        with (
            tc.tile_pool(name="const", bufs=1) as cpool,
            tc.tile_pool(name="sb", bufs=2) as sb,
            tc.tile_pool(name="dram", bufs=1, space="DRAM") as dpool,
            tc.tile_pool(name="psX", bufs=2, space="PSUM") as psX,
            tc.tile_pool(name="psAgg", bufs=2, space="PSUM") as psAgg,
        ):
            # ---- constants ----
            W1b = cpool.tile([IN, HID], BF16)
            nc.gpsimd.dma_start(out=W1b[:], in_=W1[:])
            W2b = cpool.tile([HID, OUT], BF16)
            nc.gpsimd.dma_start(out=W2b[:], in_=W2[:])
            b1t = cpool.tile([P, HID], F32)
            nc.sync.dma_start(out=b1t[:], in_=b1r[:])
            b2t = cpool.tile([P, OUT], F32)
            nc.sync.dma_start(out=b2t[:], in_=b2r[:])
            iota_t = cpool.tile([P, P], BF16)
            nc.sync.dma_start(out=iota_t[:], in_=iota[:])
            idx_t = cpool.tile([P, TOTB * 8], I16)
            nc.sync.dma_start(out=idx_t[:], in_=idx[:])
            colv_t = cpool.tile([P, NSCOL], BF16)
            nc.sync.dma_start(out=colv_t[:], in_=colv[:])
            ident = cpool.tile([P, P], BF16)
            nc.sync.dma_start(out=ident[:], in_=identd[:])

            dego_i = cpool.tile([P, NG], I32)
            nc.sync.dma_start(out=dego_i[:], in_=dego[:])
            dego_f = cpool.tile([P, NG], F32)
            nc.vector.tensor_copy(out=dego_f[:], in_=dego_i[:])
            dsq = cpool.tile([P, NG], F32)
            nc.scalar.activation(dsq[:], dego_f[:], AF.Sqrt)
            dinv = cpool.tile([P, NG], F32)
            nc.vector.reciprocal(out=dinv[:], in_=dsq[:])

            # local persistent tables for self-loop terms
            t1keep = cpool.tile([P, NG, HID], BF16)   # table1 local (rm)
            y1keep = cpool.tile([P, NG, HID], BF16)   # table2 local (rm)

            # ---- phase A: table1 = dinv * (x @ W1), rm blocks ----
            t1in = dpool.tile([PPC, HID], BF16)
            table1 = dpool.tile([NPAD, HID], BF16)
            if stage >= 1:
                XC = 14
                for c0 in range(0, NG, XC):
                    xbf = sb.tile([P, XC * P], BF16, tag="xbf", name="xbf")
                    nc.sync.dma_start(out=xbf[:], in_=xT[:, c0 * P : (c0 + XC) * P])
                    for b in range(XC):
                        ps = psX.tile([P, HID], F32, tag="px", name="ps")
                        nc.tensor.matmul(
                            out=ps[:],
                            lhsT=xbf[:, b * P : (b + 1) * P],
                            rhs=W1b[:],
                            start=True,
                            stop=True,
                        )
                        nc.scalar.activation(
                            t1keep[:, c0 + b, :], ps[:], AF.Copy,
                            scale=dinv[:, c0 + b : c0 + b + 1],
                        )
                    nc.sync.dma_start(
                        out=t1in.rearrange("(n p) f -> p n f", p=P)[:, c0 : c0 + XC, :],
                        in_=t1keep[:, c0 : c0 + XC, :],
                    )
            if stage >= 2:
                nc.gpsimd.collective_compute(
                    "AllGather", ALU.bypass, ins=[t1in[:]], outs=[table1[:]],
                    replica_groups=[list(range(NCORES))],
                )

            agin = dpool.tile([PPC, HID], BF16)
            table2 = dpool.tile([NPAD, HID], BF16)

            def agg_layer(table, orientation, epilogue, ncalls):
                # banks per supergroup
                cur_sgi = -1
                banks = []
                for ci, (sgi, ch, blk_off, nb, scol_off, nscol) in enumerate(
                    calls[:ncalls]
                ):
                    gs, ng = sgs[sgi]
                    if sgi != cur_sgi:
                        # close previous sg
                        if cur_sgi >= 0:
                            pgs, png = sgs[cur_sgi]
                            for gl in range(png):
                                epilogue(
                                    cur_sgi, pgs + gl, gl, png,
                                    banks[gl // 4][:, (gl % 4) * P : (gl % 4 + 1) * P],
                                )
                        nbank = -(-ng // 4)
                        banks = [
                            psAgg.tile(
                                [P, 512], F32, name=f"bank{i}", tag=f"aggbank{i}",
                                bufs=2,
                            )
                            for i in range(nbank)
                        ]
                        for bk in banks:
                            nc.vector.memset(bk[:], 0.0)
                        cur_sgi = sgi
                    msgs = sb.tile([P, nb, HID], BF16, tag="msgs", bufs=6, name="msgs")
                    nc.gpsimd.dma_gather(
                        msgs[:],
                        table[ch * CHUNK : (ch + 1) * CHUNK, :],
                        idx_t[:, blk_off * 8 : (blk_off + nb) * 8],
                        nb * P,
                        nb * P,
                        HID,
                        single_packet=False,
                    )
                    S = sb.tile([P, nscol, P], BF16, tag="S", bufs=6, name="S")
                    nc.vector.tensor_tensor(
                        out=S[:],
                        in0=colv_t[:, scol_off : scol_off + nscol, None].to_broadcast(
                            [P, nscol, P]
                        ),
                        in1=iota_t[:, None, :].to_broadcast([P, nscol, P]),
                        op=ALU.is_equal,
                    )
                    for j, (kk, gl, is_first, is_last) in enumerate(callcols[ci]):
                        region = banks[gl // 4][:, (gl % 4) * P : (gl % 4 + 1) * P]
                        if orientation == 1:
                            nc.tensor.matmul(
                                out=region, lhsT=S[:, j, :], rhs=msgs[:, kk, :],
                                start=False, stop=is_last, skip_group_check=True,
                            )
                        else:
                            nc.tensor.matmul(
                                out=region, lhsT=msgs[:, kk, :], rhs=S[:, j, :],
                                start=False, stop=is_last, skip_group_check=True,
                            )
                # close last sg
                if cur_sgi >= 0:
                    pgs, png = sgs[cur_sgi]
                    for gl in range(png):
                        epilogue(
                            cur_sgi, pgs + gl, gl, png,
                            banks[gl // 4][:, (gl % 4) * P : (gl % 4 + 1) * P],
                        )

            # ---- ablation stages (100+): epilogue-less agg passes ----
            if stage >= 100:
                noepi = lambda sgi, g, gl, ng, region: None
                agg_layer(table1, 1, noepi, len(calls))
                if stage == 102:
                    agg_layer(table1, 1, noepi, len(calls))
                elif stage == 103:
                    agg_layer(table1, 2, noepi, len(calls))
                stage = -1  # skip the rest

            # ---- L1 ----
            # selfb1 = dinv^2 * t1keep + b1 (in place), prepared after phase A
            if stage >= 3:
                for g in range(NG):
                    nc.vector.tensor_scalar(
                        out=t1keep[:, g, :], in0=t1keep[:, g, :],
                        scalar1=dinv[:, g : g + 1], scalar2=0.0,
                        op0=ALU.mult, op1=ALU.add,
                    )
                for g in range(NG):
                    nc.vector.tensor_tensor(
                        out=t1keep[:, g, :], in0=t1keep[:, g, :], in1=b1t[:],
                        op=ALU.add,
                    )

            def epi1(sgi, g, gl, ng, region):
                tmp = sb.tile([P, HID], F32, tag="epi1a", bufs=2, name="tmp")
                nc.scalar.activation(tmp[:], region, AF.Copy, scale=dinv[:, g : g + 1])
                tmp2 = sb.tile([P, HID], F32, tag="epi1b", bufs=2, name="tmp2")
                nc.vector.tensor_tensor(
                    out=tmp2[:], in0=tmp[:], in1=t1keep[:, g, :], op=ALU.add
                )
                nc.scalar.activation(
                    y1keep[:, g, :], tmp2[:], AF.Relu, scale=dinv[:, g : g + 1]
                )
                if gl == ng - 1:
                    gs = g - gl
                    nc.sync.dma_start(
                        out=agin.rearrange("(n p) f -> p n f", p=P)[:, gs : gs + ng, :],
                        in_=y1keep[:, gs : gs + ng, :],
                    )

            if stage >= 3:
                ncalls1 = len(calls) if stage > 3 else max(
                    i for i, c in enumerate(calls) if c[0] == 0
                ) + 1
                agg_layer(table1, 1, epi1, ncalls1)
            if stage >= 5:
                nc.gpsimd.collective_compute(
                    "AllGather", ALU.bypass, ins=[agin[:]], outs=[table2[:]],
                    replica_groups=[list(range(NCORES))],
                )

            # ---- L2 ----
            # self2fm = dinv(free) * y1Tkeep (in place): scale along free dim
            # dinv as row per group: use tensor_scalar? dinv varies along free
            # (targets) -> use tensor_tensor with broadcast of a [1, P] slice.
            outsg = {}

            def epi2(sgi, g, gl, ng, region):
                if gl == 0:
                    outsg[sgi] = sb.tile([P, ng, OUT], F32, name="outs", tag="outs", bufs=2)
                a3 = sb.tile([P, HID], BF16, tag="a3", bufs=2, name="a3")
                nc.vector.tensor_tensor(
                    out=a3[:], in0=region, in1=y1keep[:, g, :], op=ALU.add
                )
                pst = psX.tile([P, P], BF16, tag="ptr", name="pst", bufs=1)
                nc.tensor.transpose(pst[:], a3[:], ident[:])
                a4 = sb.tile([HID, P], BF16, tag="a4", bufs=2, name="a4")
                nc.scalar.activation(a4[:], pst[:], AF.Copy)
                psf = psX.tile([P, OUT], F32, tag="px2", name="psf", bufs=1)
                nc.tensor.matmul(out=psf[:], lhsT=a4[:], rhs=W2b[:], start=True, stop=True)
                tmp = sb.tile([P, OUT], F32, tag="epi2a", bufs=2, name="tmp3")
                nc.scalar.activation(tmp[:], psf[:], AF.Copy, scale=dinv[:, g : g + 1])
                nc.vector.tensor_tensor(
                    out=outsg[sgi][:, gl, :], in0=tmp[:], in1=b2t[:], op=ALU.add
                )
                if gl == ng - 1:
                    gs = g - gl
                    nc.sync.dma_start(out=y[:, gs : gs + ng, :], in_=outsg[sgi][:])

            if stage >= 6:
                agg_layer(table2, 1, epi2, len(calls))

    nc.finalize()
    return nc


def make_in_maps(inputs, sched, idx_list, colv_list, dego_list):
    x = np.asarray(inputs["x"], np.float32)
    W1 = np.asarray(inputs["W1"], np.float32)
    W2 = np.asarray(inputs["W2"], np.float32)
    b1 = np.asarray(inputs["b1"], np.float32)
    b2 = np.asarray(inputs["b2"], np.float32)
    iota_np = np.tile(np.arange(P, dtype=ml_dtypes.bfloat16)[None, :], (P, 1))
    ident_np = np.eye(P, dtype=ml_dtypes.bfloat16)
    b1r = np.tile(b1[None, :], (P, 1)).astype(np.float32)
    b2r = np.tile(b2[None, :], (P, 1)).astype(np.float32)
    gids = sched["gids"]
    xp = np.zeros((NPAD, P), np.float32)
    xp[gids] = x
    # dinvrow[g, t] = deg(node at slot g*P+t of this core)^-1/2 -- per core
    in_maps = []
    for c in range(NCORES):
        xs = np.ascontiguousarray(xp[c * PPC : (c + 1) * PPC].T.astype(ml_dtypes.bfloat16))
        in_maps.append(
            {
                "xT": xs,
                "dego": dego_list[c],
                "W1": W1,
                "W2": W2,
                "b1r": b1r,
                "b2r": b2r,
                "iota": iota_np,
                "identd": ident_np,
                "idx": idx_list[c],
                "colv": colv_list[c],
            }
        )
    return in_maps


def assemble_output(results, sched):
    outs = []
    for c in range(NCORES):
        yc = results[c]["y"]
        yc = np.transpose(yc, (1, 0, 2)).reshape(PPC, OUT)
        outs.append(yc)
    Y = np.concatenate(outs, axis=0)
    return np.ascontiguousarray(Y[sched["gids"]])


def kernel(**inputs):
    sched, idx_list, colv_list, dego_list, _ = host_prep(inputs["edge_index"])
    nc = build_kernel(sched)
    in_maps = make_in_maps(inputs, sched, idx_list, colv_list, dego_list)
    res = run_bass_kernel_spmd(nc, in_maps, core_ids=list(range(NCORES)))
    return assemble_output(res.results, sched)



# revision 5
# speedup vs baseline: 2.3110x; 2.3110x over previous
"""GCN forward on 8 TRN2 NeuronCores — v2.

Changes vs baseline:
  - Degree-balanced node->(core,group) assignment (snake by in-degree) so
    per-(core,group,chunk) segment counts are near-equal across cores.
  - Self-loops removed from the gather stream; their contribution is added
    in the epilogues from SBUF-resident local tables.
  - Packed block layout: per (supergroup, chunk), group segments are laid
    out back-to-back at fixed slot offsets (max-over-core length), so
    128-message blocks can span group boundaries. Boundary blocks issue one
    matmul per intersecting group using a separate one-hot column.

Math (reference.py): 2-layer GCN, N=100000, E=1600000, IN=HID=128, OUT=64,
symmetric normalization with self-loops:
  deg[t] = in_degree(t)+1 ; dinv = deg^-1/2
  table1[s] = dinv[s]*(x@W1)[s] ; y1[t] = relu(dinv[t]*(sum_e table1[row_e]
               + dinv[t]*table1[t]) + b1)
  table2[s] = dinv[s]*y1[s]     ; out[t] = (dinv[t]*(sum_e table2[row_e]
               + dinv[t]*table2[t]))@W2 + b2
"""

import sys

sys.path.insert(0, "/opt/trn_rl_repo")
import numpy as np
import ml_dtypes

import concourse.bass as bass
import concourse.mybir as mybir
import concourse.tile as tile
from concourse import bacc
from concourse.bass_utils import run_bass_kernel_spmd

F32 = mybir.dt.float32
BF16 = mybir.dt.bfloat16
I16 = mybir.dt.int16
I32 = mybir.dt.int32
AF = mybir.ActivationFunctionType
ALU = mybir.AluOpType

P = 128
N, E = 100000, 1600000
IN, HID, OUT = 128, 128, 64
NCORES = 8
PPC = 12544
NPAD = PPC * NCORES
NCHUNK = 4
CHUNK = NPAD // NCHUNK
NG = PPC // P  # 98 groups/core
SG = 8         # groups per supergroup
MAX_CALL_BLOCKS = 24


def host_prep(edge_index):
    row = np.asarray(edge_index[0], dtype=np.int64)
    col = np.asarray(edge_index[1], dtype=np.int64)
    deg = np.bincount(col, minlength=N).astype(np.int64) + 1

    # --- degree-balanced snake assignment of nodes to (core,group) bins ---
    NBINS = NCORES * NG
    order = np.argsort(-deg, kind="stable")
    bin_of = np.empty(N, np.int64)
    idx0 = 0
    rounds = 0
    while idx0 < N:
        take = min(NBINS, N - idx0)
        sel = order[idx0 : idx0 + take]
        seq = np.arange(take) if rounds % 2 == 0 else (NBINS - 1 - np.arange(take))
        bin_of[sel] = seq
        idx0 += take
        rounds += 1
    # slots within bins (max P per bin guaranteed: ceil(N/NBINS) <= P)
    order2 = np.argsort(bin_of, kind="stable")
    bcounts = np.bincount(bin_of, minlength=NBINS)
    bstarts = np.zeros(NBINS, np.int64)
    bstarts[1:] = np.cumsum(bcounts)[:-1]
    slot_of = np.empty(N, np.int64)
    slot_of[order2] = np.arange(N, dtype=np.int64) - np.repeat(bstarts, bcounts)
    assert bcounts.max() <= P
    core_of = bin_of // NG
    grp_of = bin_of % NG
    gids = core_of * PPC + grp_of * P + slot_of  # node -> padded slot

    # --- edges (no self-loops in the stream) ---
    grow = gids[row]
    gcol = gids[col]
    owner = gcol // PPC
    g_all = (gcol - owner * PPC) >> 7
    loc_all = (gcol - owner * PPC) & 127
    ch_all = grow // CHUNK
    lidx_all = (grow % CHUNK).astype(np.int64)

    NSEG = NG * NCHUNK
    counts = np.zeros((NCORES, NSEG), np.int64)
    per_core = []
    for c in range(NCORES):
        sel = owner == c
        g = g_all[sel]
        ch = ch_all[sel]
        o = np.lexsort((ch, g))
        seg = (g * NCHUNK + ch)[o]
        counts[c] = np.bincount(seg, minlength=NSEG)
        per_core.append((seg, lidx_all[sel][o], loc_all[sel][o]))

    seg_len = counts.max(axis=0)  # fixed per-(g,ch) slot allocation
    seg_len = np.maximum(seg_len, 1)

    # --- packed layout: per (sg, ch) concatenate group segments ---
    sgs = []
    g0 = 0
    while g0 < NG:
        sgs.append((g0, min(SG, NG - g0)))
        g0 += SG

    seg_off = np.zeros(NSEG, np.int64)   # message-slot offset of each segment
    calls = []                            # (sgi, ch, blk_off, nblk, scol_off, nscol)
    # scol list entries: (block_k_within_call, gl, colv_col_index, is_last)
    callcols = []
    tot_blocks = 0
    tot_scols = 0
    for sgi, (gs, ng) in enumerate(sgs):
        for ch in range(NCHUNK):
            # slots for this (sg, ch)
            L = 0
            seg_bounds = []
            for gl in range(ng):
                s = (gs + gl) * NCHUNK + ch
                seg_off[s] = tot_blocks * P + L
                seg_bounds.append((L, L + seg_len[s]))
                L += seg_len[s]
            nblk = -(-L // P)
            # columns: for each block, each group intersecting it
            cols = []
            for k in range(nblk):
                blo, bhi = k * P, (k + 1) * P
                for gl in range(ng):
                    slo, shi = seg_bounds[gl]
                    if slo < bhi and shi > blo:
                        is_first_seg = ch == 0 and slo >= blo
                        is_last_seg = ch == NCHUNK - 1 and shi <= bhi
                        cols.append((k, gl, is_first_seg, is_last_seg))
            # split into calls of <= MAX_CALL_BLOCKS blocks
            k0 = 0
            while k0 < nblk:
                nb = min(MAX_CALL_BLOCKS, nblk - k0)
                ccols = [
                    (k - k0, gl, first, last)
                    for (k, gl, first, last) in cols
                    if k0 <= k < k0 + nb
                ]
                calls.append(
                    (sgi, ch, tot_blocks + k0, nb, tot_scols, len(ccols))
                )
                callcols.append(ccols)
                tot_scols += len(ccols)
                k0 += nb
            tot_blocks += nblk
    TOTB = tot_blocks
    NSCOL = tot_scols

    # --- per-core idx / colv arrays ---
    idx_list, colv_list = [], []
    for c in range(NCORES):
        seg, lidx, loc = per_core[c]
        seg_start = np.zeros(NSEG, np.int64)
        seg_start[1:] = np.cumsum(counts[c])[:-1]
        rank = np.arange(seg.shape[0]) - seg_start[seg]
        dest = seg_off[seg] + rank
        idx_arr = np.zeros(TOTB * P, np.int16)
        idx_arr[dest] = lidx.astype(np.int16)
        idx_list.append(np.tile(idx_arr.reshape(-1, 16).T, (8, 1)).copy())

    for c in range(NCORES):
        seg, lidx, loc = per_core[c]
        seg_start = np.zeros(NSEG, np.int64)
        seg_start[1:] = np.cumsum(counts[c])[:-1]
        rank = np.arange(seg.shape[0]) - seg_start[seg]
        dest = seg_off[seg] + rank
        slot_gl = np.full(TOTB * P, -1, np.int64)
        slot_loc = np.full(TOTB * P, -1, np.int64)
        slot_gl[dest] = (seg // NCHUNK) % NG  # group index g (global)
        slot_loc[dest] = loc
        colv_arr = np.full((NSCOL, P), -1.0, np.float32)
        for ci, (sgi, ch, blk_off, nb, scol_off, nscol) in enumerate(calls):
            gs, ng = sgs[sgi]
            for j, (kk, gl, first, last) in enumerate(callcols[ci]):
                blk = blk_off + kk
                sl = slice(blk * P, (blk + 1) * P)
                m = slot_gl[sl] == gs + gl
                cv = colv_arr[scol_off + j]
                cv[m] = slot_loc[sl][m].astype(np.float32)
        colv_list.append(
            np.ascontiguousarray(colv_arr.T.astype(ml_dtypes.bfloat16))
        )

    degp = np.ones(NPAD, np.int32)
    degp[gids] = deg.astype(np.int32)
    dego_list = [
        np.ascontiguousarray(degp[c * PPC : (c + 1) * PPC].reshape(NG, P).T)
        for c in range(NCORES)
    ]

    sched = {
        "sgs": sgs,
        "calls": calls,
        "callcols": callcols,
        "TOTB": TOTB,
        "NSCOL": NSCOL,
        "gids": gids,
    }
    return sched, idx_list, colv_list, dego_list, gids


def build_kernel(sched, stage=9):
    sgs, calls, callcols = sched["sgs"], sched["calls"], sched["callcols"]
    TOTB, NSCOL = sched["TOTB"], sched["NSCOL"]

    nc = bacc.Bacc("TRN2", target_bir_lowering=False, num_devices=NCORES)
    xT = nc.dram_tensor("xT", [P, PPC], BF16, kind="ExternalInput")
    dego = nc.dram_tensor("dego", [P, NG], I32, kind="ExternalInput")
    W1 = nc.dram_tensor("W1", [IN, HID], F32, kind="ExternalInput")
    W2 = nc.dram_tensor("W2", [HID, OUT], F32, kind="ExternalInput")
    b1r = nc.dram_tensor("b1r", [P, HID], F32, kind="ExternalInput")
    b2r = nc.dram_tensor("b2r", [P, OUT], F32, kind="ExternalInput")
    iota = nc.dram_tensor("iota", [P, P], BF16, kind="ExternalInput")
    identd = nc.dram_tensor("identd", [P, P], BF16, kind="ExternalInput")
    idx = nc.dram_tensor("idx", [P, TOTB * 8], I16, kind="ExternalInput")
    colv = nc.dram_tensor("colv", [P, NSCOL], BF16, kind="ExternalInput")
    y = nc.dram_tensor("y", [P, NG, OUT], F32, kind="ExternalOutput")

    with tile.TileContext(nc) as tc:
        with (
            tc.tile_pool(name="const", bufs=1) as cpool,
            tc.tile_pool(name="sb", bufs=2) as sb,
            tc.tile_pool(name="dram", bufs=1, space="DRAM") as dpool,
            tc.tile_pool(name="psX", bufs=2, space="PSUM") as psX,
            tc.tile_pool(name="psAgg", bufs=2, space="PSUM") as psAgg,
        ):
            # ---- constants ----
            W1b = cpool.tile([IN, HID], BF16)
            nc.gpsimd.dma_start(out=W1b[:], in_=W1[:])
            W2b = cpool.tile([HID, OUT], BF16)
            nc.gpsimd.dma_start(out=W2b[:], in_=W2[:])
            b1t = cpool.tile([P, HID], F32)
            nc.sync.dma_start(out=b1t[:], in_=b1r[:])
            b2t = cpool.tile([P, OUT], F32)
            nc.sync.dma_start(out=b2t[:], in_=b2r[:])
            iota_t = cpool.tile([P, P], BF16)
            nc.sync.dma_start(out=iota_t[:], in_=iota[:])
            idx_t = cpool.tile([P, TOTB * 8], I16)
            nc.sync.dma_start(out=idx_t[:], in_=idx[:])
            colv_t = cpool.tile([P, NSCOL], BF16)
            nc.sync.dma_start(out=colv_t[:], in_=colv[:])
            ident = cpool.tile([P, P], BF16)
            nc.sync.dma_start(out=ident[:], in_=identd[:])

            dego_i = cpool.tile([P, NG], I32)
            nc.sync.dma_start(out=dego_i[:], in_=dego[:])
            dego_f = cpool.tile([P, NG], F32)
            nc.vector.tensor_copy(out=dego_f[:], in_=dego_i[:])
            dsq = cpool.tile([P, NG], F32)
            nc.scalar.activation(dsq[:], dego_f[:], AF.Sqrt)
            dinv = cpool.tile([P, NG], F32)
            nc.vector.reciprocal(out=dinv[:], in_=dsq[:])

            # local persistent tables for self-loop terms
            t1keep = cpool.tile([P, NG, HID], BF16)   # table1 local (rm)
            y1keep = cpool.tile([P, NG, HID], BF16)   # table2 local (rm)

            # ---- phase A: table1 = dinv * (x @ W1), rm blocks ----
            t1in = dpool.tile([PPC, HID], BF16)
            table1 = dpool.tile([NPAD, HID], BF16, addr_space="Shared")
            if stage >= 1:
                XC = 14
                for c0 in range(0, NG, XC):
                    xbf = sb.tile([P, XC * P], BF16, tag="xbf", name="xbf")
                    nc.sync.dma_start(out=xbf[:], in_=xT[:, c0 * P : (c0 + XC) * P])
                    for b in range(XC):
                        ps = psX.tile([P, HID], F32, tag="px", name="ps")
                        nc.tensor.matmul(
                            out=ps[:],
                            lhsT=xbf[:, b * P : (b + 1) * P],
                            rhs=W1b[:],
                            start=True,
                            stop=True,
                        )
                        nc.scalar.activation(
                            t1keep[:, c0 + b, :], ps[:], AF.Copy,
                            scale=dinv[:, c0 + b : c0 + b + 1],
                        )
                    nc.sync.dma_start(
                        out=t1in.rearrange("(n p) f -> p n f", p=P)[:, c0 : c0 + XC, :],
                        in_=t1keep[:, c0 : c0 + XC, :],
                    )
            if stage >= 2:
                nc.gpsimd.collective_compute(
                    "AllGather", ALU.bypass, ins=[t1in[:]], outs=[table1[:]],
                    replica_groups=[list(range(NCORES))],
                )

            agin = dpool.tile([PPC, HID], BF16)
            table2 = dpool.tile([NPAD, HID], BF16, addr_space="Shared")

            def agg_layer(table, orientation, epilogue, ncalls):
                # banks per supergroup
                cur_sgi = -1
                banks = []
                for ci, (sgi, ch, blk_off, nb, scol_off, nscol) in enumerate(
                    calls[:ncalls]
                ):
                    gs, ng = sgs[sgi]
                    if sgi != cur_sgi:
                        # close previous sg
                        if cur_sgi >= 0:
                            pgs, png = sgs[cur_sgi]
                            for gl in range(png):
                                epilogue(
                                    cur_sgi, pgs + gl, gl, png,
                                    banks[gl // 4][:, (gl % 4) * P : (gl % 4 + 1) * P],
                                )
                        nbank = -(-ng // 4)
                        banks = [
                            psAgg.tile(
                                [P, 512], F32, name=f"bank{i}", tag=f"aggbank{i}",
                                bufs=2,
                            )
                            for i in range(nbank)
                        ]
                        for bk in banks:
                            nc.vector.memset(bk[:], 0.0)
                        cur_sgi = sgi
                    msgs = sb.tile([P, nb, HID], BF16, tag="msgs", bufs=6, name="msgs")
                    nc.gpsimd.dma_gather(
                        msgs[:],
                        table[ch * CHUNK : (ch + 1) * CHUNK, :],
                        idx_t[:, blk_off * 8 : (blk_off + nb) * 8],
                        nb * P,
                        nb * P,
                        HID,
                        single_packet=False,
                    )
                    S = sb.tile([P, nscol, P], BF16, tag="S", bufs=6, name="S")
                    nc.vector.tensor_tensor(
                        out=S[:],
                        in0=colv_t[:, scol_off : scol_off + nscol, None].to_broadcast(
                            [P, nscol, P]
                        ),
                        in1=iota_t[:, None, :].to_broadcast([P, nscol, P]),
                        op=ALU.is_equal,
                    )
                    for j, (kk, gl, is_first, is_last) in enumerate(callcols[ci]):
                        region = banks[gl // 4][:, (gl % 4) * P : (gl % 4 + 1) * P]
                        if orientation == 1:
                            nc.tensor.matmul(
                                out=region, lhsT=S[:, j, :], rhs=msgs[:, kk, :],
                                start=False, stop=is_last, skip_group_check=True,
                            )
                        else:
                            nc.tensor.matmul(
                                out=region, lhsT=msgs[:, kk, :], rhs=S[:, j, :],
                                start=False, stop=is_last, skip_group_check=True,
                            )
                # close last sg
                if cur_sgi >= 0:
                    pgs, png = sgs[cur_sgi]
                    for gl in range(png):
                        epilogue(
                            cur_sgi, pgs + gl, gl, png,
                            banks[gl // 4][:, (gl % 4) * P : (gl % 4 + 1) * P],
                        )

            # ---- ablation stages (100+): epilogue-less agg passes ----
            if stage >= 100:
                noepi = lambda sgi, g, gl, ng, region: None
                agg_layer(table1, 1, noepi, len(calls))
                if stage == 102:
                    agg_layer(table1, 1, noepi, len(calls))
                elif stage == 103:
                    agg_layer(table1, 2, noepi, len(calls))
                stage = -1  # skip the rest

            # ---- L1 ----
            # selfb1 = dinv^2 * t1keep + b1 (in place), prepared after phase A
            if stage >= 3:
                for g in range(NG):
                    nc.vector.tensor_scalar(
                        out=t1keep[:, g, :], in0=t1keep[:, g, :],
                        scalar1=dinv[:, g : g + 1], scalar2=0.0,
                        op0=ALU.mult, op1=ALU.add,
                    )
                for g in range(NG):
                    nc.vector.tensor_tensor(
                        out=t1keep[:, g, :], in0=t1keep[:, g, :], in1=b1t[:],
                        op=ALU.add,
                    )

            def epi1(sgi, g, gl, ng, region):
                tmp = sb.tile([P, HID], F32, tag="epi1a", bufs=2, name="tmp")
                nc.scalar.activation(tmp[:], region, AF.Copy, scale=dinv[:, g : g + 1])
                tmp2 = sb.tile([P, HID], F32, tag="epi1b", bufs=2, name="tmp2")
                nc.vector.tensor_tensor(
                    out=tmp2[:], in0=tmp[:], in1=t1keep[:, g, :], op=ALU.add
                )
                nc.scalar.activation(
                    y1keep[:, g, :], tmp2[:], AF.Relu, scale=dinv[:, g : g + 1]
                )
                if gl == ng - 1:
                    gs = g - gl
                    nc.sync.dma_start(
                        out=agin.rearrange("(n p) f -> p n f", p=P)[:, gs : gs + ng, :],
                        in_=y1keep[:, gs : gs + ng, :],
                    )

            if stage >= 3:
                ncalls1 = len(calls) if stage > 3 else max(
                    i for i, c in enumerate(calls) if c[0] == 0
                ) + 1
                agg_layer(table1, 1, epi1, ncalls1)
            if stage >= 5:
                nc.gpsimd.collective_compute(
                    "AllGather", ALU.bypass, ins=[agin[:]], outs=[table2[:]],
                    replica_groups=[list(range(NCORES))],
                )

            # ---- L2 ----
            # self2fm = dinv(free) * y1Tkeep (in place): scale along free dim
            # dinv as row per group: use tensor_scalar? dinv varies along free
            # (targets) -> use tensor_tensor with broadcast of a [1, P] slice.
            outsg = {}

            def epi2(sgi, g, gl, ng, region):
                if gl == 0:
                    outsg[sgi] = sb.tile([P, ng, OUT], F32, name="outs", tag="outs", bufs=2)
                a3 = sb.tile([P, HID], BF16, tag="a3", bufs=2, name="a3")
                nc.vector.tensor_tensor(
                    out=a3[:], in0=region, in1=y1keep[:, g, :], op=ALU.add
                )
                pst = psX.tile([P, P], BF16, tag="ptr", name="pst", bufs=1)
                nc.tensor.transpose(pst[:], a3[:], ident[:])
                a4 = sb.tile([HID, P], BF16, tag="a4", bufs=2, name="a4")
                nc.scalar.activation(a4[:], pst[:], AF.Copy)
                psf = psX.tile([P, OUT], F32, tag="px2", name="psf", bufs=1)
                nc.tensor.matmul(out=psf[:], lhsT=a4[:], rhs=W2b[:], start=True, stop=True)
                tmp = sb.tile([P, OUT], F32, tag="epi2a", bufs=2, name="tmp3")
                nc.scalar.activation(tmp[:], psf[:], AF.Copy, scale=dinv[:, g : g + 1])
                nc.vector.tensor_tensor(
                    out=outsg[sgi][:, gl, :], in0=tmp[:], in1=b2t[:], op=ALU.add
                )
                if gl == ng - 1:
                    gs = g - gl
                    nc.sync.dma_start(out=y[:, gs : gs + ng, :], in_=outsg[sgi][:])

            if stage >= 6:
                agg_layer(table2, 1, epi2, len(calls))

    nc.finalize()
    return nc


def make_in_maps(inputs, sched, idx_list, colv_list, dego_list):
    x = np.asarray(inputs["x"], np.float32)
    W1 = np.asarray(inputs["W1"], np.float32)
    W2 = np.asarray(inputs["W2"], np.float32)
    b1 = np.asarray(inputs["b1"], np.float32)
    b2 = np.asarray(inputs["b2"], np.float32)
    iota_np = np.tile(np.arange(P, dtype=ml_dtypes.bfloat16)[None, :], (P, 1))
    ident_np = np.eye(P, dtype=ml_dtypes.bfloat16)
    b1r = np.tile(b1[None, :], (P, 1)).astype(np.float32)
    b2r = np.tile(b2[None, :], (P, 1)).astype(np.float32)
    gids = sched["gids"]
    xp = np.zeros((NPAD, P), np.float32)
    xp[gids] = x
    # dinvrow[g, t] = deg(node at slot g*P+t of this core)^-1/2 -- per core
    in_maps = []
    for c in range(NCORES):
        xs = np.ascontiguousarray(xp[c * PPC : (c + 1) * PPC].T.astype(ml_dtypes.bfloat16))
        in_maps.append(
            {
                "xT": xs,
                "dego": dego_list[c],
                "W1": W1,
                "W2": W2,
                "b1r": b1r,
                "b2r": b2r,
                "iota": iota_np,
                "identd": ident_np,
                "idx": idx_list[c],
                "colv": colv_list[c],
            }
        )
    return in_maps


def assemble_output(results, sched):
    outs = []
    for c in range(NCORES):
        yc = results[c]["y"]
        yc = np.transpose(yc, (1, 0, 2)).reshape(PPC, OUT)
        outs.append(yc)
    Y = np.concatenate(outs, axis=0)
    return np.ascontiguousarray(Y[sched["gids"]])


def kernel(**inputs):
    sched, idx_list, colv_list, dego_list, _ = host_prep(inputs["edge_index"])
    nc = build_kernel(sched)
    in_maps = make_in_maps(inputs, sched, idx_list, colv_list, dego_list)
    res = run_bass_kernel_spmd(nc, in_maps, core_ids=list(range(NCORES)))
    return assemble_output(res.results, sched)

